# revision 6
# baseline (speedup 1.0000x reference)
"""Trainium2 Bass kernel for nn_BinaryConv2d (B=16, C=64, H=W=256, 3x3, pad 1).

Forward semantics (STE forward values):
  act = sign(x * rd_k + rd_b)                  in {-1, 0, +1}
  bw  = scaling[co] * sign(conv_w)             scaling = mean |conv_w| per out-ch
  y   = conv2d(act, bw, pad=1)
  y   = prelu(y + pr_bias0) + pr_bias1 + x     prelu slope per channel

Strategy: data-parallel over batch, 2 images per core (8 cores).  The two
images' 64 channels are stacked on the 128 SBUF partitions.  Activations are
binarized to bf16 +-1 on the Scalar engine; the 3x3 conv is 9 accumulating
PSUM matmuls with block-diagonal +-1 bf16 weights (exact integer arithmetic
in fp32 PSUM).  Per-channel scaling / PReLU / biases / residual are fused
post-ops on ScalarE / GpSimd / VectorE.
"""

import sys

if "/opt/trn_rl_repo" not in sys.path:
    sys.path.insert(0, "/opt/trn_rl_repo")

from contextlib import ExitStack

import ml_dtypes
import numpy as np

import concourse.bacc as bacc
import concourse.bass as bass
import concourse.tile as tile
from concourse import mybir
from concourse.bass_utils import run_bass_kernel_spmd

B, C, H, W = 16, 64, 256, 256
NCORES = 8
HS = 32                      # output rows per strip
NSTRIPS = H // HS
P = 128                      # partitions = 2 images x 64 channels

F32 = mybir.dt.float32
BF16 = mybir.dt.bfloat16
AF = mybir.ActivationFunctionType
ALU = mybir.AluOpType

# Param table columns (per-partition f32 scalars)
PK, PB, PS, PB0, PCM, PB1, PSL = 0, 1, 2, 3, 4, 5, 6

# The ACT-engine Lrelu activation computes something other than
# prelu(x, alpha) on TRN2 hardware (measured absmax 0.1 vs reference), so the
# PReLU is done with min/mult/add ops instead.
USE_LRELU = False

SIGN_CHUNK = 9               # rows of sign-activation per ACT instruction


def _emit(tc, nc, x_d, w_d, p_d, y_d):
    x3 = x_d.rearrange("p (h w) -> p h w", w=W)
    y3 = y_d.rearrange("p (h w) -> p h w", w=W)
    w3 = w_d.rearrange("p (j m) -> p j m", j=9)

    with ExitStack() as ctx:
        consts = ctx.enter_context(tc.tile_pool(name="consts", bufs=1))
        xpool = ctx.enter_context(tc.tile_pool(name="xpool", bufs=2))
        apool = ctx.enter_context(tc.tile_pool(name="apool", bufs=2))
        ypool = ctx.enter_context(tc.tile_pool(name="ypool", bufs=2))
        tpool = ctx.enter_context(tc.tile_pool(name="tpool", bufs=6))
        pspool = ctx.enter_context(tc.tile_pool(name="pspool", bufs=8, space="PSUM"))

        wt = consts.tile([P, 9, 128], BF16)
        nc.sync.dma_start(out=wt, in_=w3)
        pt = consts.tile([P, 8], F32)
        nc.sync.dma_start(out=pt, in_=p_d)

        for s in range(NSTRIPS):
            h0 = s * HS
            row_lo = max(h0 - 1, 0)
            row_hi = min(h0 + HS + 1, H)
            nr = row_hi - row_lo
            r0 = row_lo - (h0 - 1)          # 1 for first strip, else 0

            # x strip rows h0-1 .. h0+HS (tile row a <-> global row h0-1+a)
            xs = xpool.tile([P, HS + 2, W], F32, name="xs")
            nc.sync.dma_start(out=xs[:, r0:r0 + nr, :], in_=x3[:, row_lo:row_hi, :])

            # binarized activations, zero-padded by 1 in both spatial dims
            act = apool.tile([P, HS + 2, W + 2], BF16, name="act")
            nc.gpsimd.memset(act[:, :, 0:1], 0.0)
            nc.gpsimd.memset(act[:, :, W + 1:W + 2], 0.0)
            if s == 0:
                nc.gpsimd.memset(act[:, 0:1, :], 0.0)
            if s == NSTRIPS - 1:
                nc.gpsimd.memset(act[:, HS + 1:HS + 2, :], 0.0)
            for c0 in range(r0, r0 + nr, SIGN_CHUNK):
                c1 = min(c0 + SIGN_CHUNK, r0 + nr)
                nc.scalar.activation(
                    act[:, c0:c1, 1:W + 1], xs[:, c0:c1, :], AF.Sign,
                    bias=pt[:, PB:PB + 1], scale=pt[:, PK:PK + 1],
                )

            ys = ypool.tile([P, HS, W], F32, name="ys")
            for t in range(HS // 2):
                ps = pspool.tile([P, 2, W], F32, name="ps")
                for j in range(9):
                    kh, kw = divmod(j, 3)
                    nc.tensor.matmul(
                        ps,
                        lhsT=wt[:, j, :],
                        rhs=act[:, 2 * t + kh:2 * t + kh + 2, kw:kw + W],
                        start=(j == 0),
                        stop=(j == 8),
                    )
                u = ys[:, 2 * t:2 * t + 2, :]
                xres = xs[:, 2 * t + 1:2 * t + 3, :]
                xb = tpool.tile([P, 2, W], F32, name="xb")
                nc.vector.tensor_scalar_add(xb, xres, pt[:, PB1:PB1 + 1])
                if USE_LRELU:
                    # u = prelu(ps*scaling + b0) with per-channel slope
                    nc.scalar.activation(
                        u, ps, AF.Lrelu,
                        bias=pt[:, PB0:PB0 + 1], scale=pt[:, PS:PS + 1],
                        alpha=pt[:, PSL:PSL + 1],
                    )
                else:
                    # u = ps*scaling + b0 ; u += (slope-1)*min(u, 0)
                    nc.scalar.activation(
                        u, ps, AF.Identity,
                        bias=pt[:, PB0:PB0 + 1], scale=pt[:, PS:PS + 1],
                    )
                    m = tpool.tile([P, 2, W], F32, name="m")
                    nc.vector.tensor_scalar(
                        m, u, 0.0, pt[:, PCM:PCM + 1], ALU.min, ALU.mult
                    )
                    nc.vector.tensor_tensor(u, u, m, ALU.add)
                # u += x + b1
                nc.vector.tensor_tensor(u, u, xb, ALU.add)
            nc.sync.dma_start(out=y3[:, h0:h0 + HS, :], in_=ys)


def build_nc():
    nc = bacc.Bacc("TRN2", target_bir_lowering=False, debug=False,
                   num_devices=NCORES)
    x_d = nc.dram_tensor("xin", [P, H * W], F32, kind="ExternalInput").ap()
    w_d = nc.dram_tensor("wp", [P, 9 * 128], BF16, kind="ExternalInput").ap()
    p_d = nc.dram_tensor("pp", [P, 8], F32, kind="ExternalInput").ap()
    y_d = nc.dram_tensor("yout", [P, H * W], F32, kind="ExternalOutput").ap()
    with tile.TileContext(nc) as tc:
        _emit(tc, nc, x_d, w_d, p_d, y_d)
    nc.compile()
    return nc


_NC_CACHE = []


def _get_nc():
    if not _NC_CACHE:
        _NC_CACHE.append(build_nc())
    return _NC_CACHE[0]


def make_inputs(x, rd_k, rd_b, beta, conv_w, pr_bias0, prelu_w, pr_bias1):
    """Host-side prep: per-channel param table, packed sign weights, shards."""
    k = np.asarray(rd_k, np.float32).reshape(C)
    b = np.asarray(rd_b, np.float32).reshape(C)
    s = np.mean(np.abs(np.asarray(conv_w, np.float32)), axis=(1, 2, 3))
    b0 = np.asarray(pr_bias0, np.float32).reshape(C)
    slope = np.asarray(prelu_w, np.float32).reshape(C)
    b1 = np.asarray(pr_bias1, np.float32).reshape(C)
    cm = slope - 1.0
    cols = np.stack([k, b, s, b0, cm, b1, slope, np.zeros(C, np.float32)], axis=1)
    pp = np.concatenate([cols, cols], axis=0).astype(np.float32)  # [128, 8]

    sw = np.sign(np.asarray(conv_w, np.float32)).astype(np.float32)  # [co,ci,kh,kw]
    wp = np.zeros((P, 9, 128), np.float32)
    for j in range(9):
        kh, kw = divmod(j, 3)
        S = sw[:, :, kh, kw].T  # [ci, co]
        wp[0:C, j, 0:C] = S
        wp[C:P, j, C:P] = S
    wp = np.ascontiguousarray(wp.reshape(P, 9 * 128)).astype(ml_dtypes.bfloat16)

    x = np.asarray(x, np.float32)
    in_maps = []
    for c in range(NCORES):
        xc = np.ascontiguousarray(x[2 * c:2 * c + 2]).reshape(P, H * W)
        in_maps.append({"xin": xc, "wp": wp, "pp": pp})
    return in_maps


def kernel(x, rd_k, rd_b, beta, conv_w, pr_bias0, prelu_w, pr_bias1):
    in_maps = make_inputs(x, rd_k, rd_b, beta, conv_w, pr_bias0, prelu_w,
                          pr_bias1)
    nc = _get_nc()
    res = run_bass_kernel_spmd(nc, in_maps, core_ids=list(range(NCORES)))
    y = np.empty((B, C, H, W), np.float32)
    for c in range(NCORES):
        y[2 * c:2 * c + 2] = res.results[c]["yout"].reshape(2, C, H, W)
    return y
